# revision 22
# baseline (speedup 1.0000x reference)
"""DiffLogicLayer forward on 8 TRN2 NeuronCores — gate-sharded, bf16.

Math: every one of the 16 soft logic ops is affine in {1, a, b, a*b}, so
    out[n, o] = C0[o] + C1[o]*a + C2[o]*b + C3[o]*a*b
with a = x[n, conn_a[o]], b = x[n, conn_b[o]] and C = softmax(weights) @ M
for the constant 16x4 matrix M of op coefficients (host-precomputed; it is
O(out_dim) work).

Sharding: out_dim (gate axis) split 8 ways; each core owns 1024 gates and
the full 4096-batch. Host supplies xT = x.T cast to bf16 so each gathered
operand row is 8 KiB contiguous — dma_gather descriptor generation on the
Q7 costs ~8 ns/descriptor of Pool-engine time, so few/large descriptors
(2048 x 8 KiB per core) beat many/small ones. Gates land on partitions,
256 per gather (2 slots); per 128-gate slot:
    u = C3*a + C2   (DVE tensor_scalar, per-partition scalars, bf16 4x)
    w = C1*a + C0   (ACT identity, per-partition scale/bias)
    u = u * b       (DVE, bf16 2x)
    o = u + w       (DVE, bf16 2x)
and the [128, 4096] bf16 slot is DMAd contiguously to outT. Host casts the
per-core [1024, 4096] bf16 shards back to f32 and transposes/concats.
"""

import numpy as np
from contextlib import ExitStack

import concourse.bacc as bacc
import concourse.mybir as mybir
import concourse.tile as tile
from concourse.bass_utils import run_bass_kernel_spmd

N_CORES = 8
BATCH, IN_DIM, OUT_DIM = 4096, 4096, 8192
GPC = OUT_DIM // N_CORES          # gates per core = 1024
SLOTS = GPC // 128                # 128-gate slots per core = 8
GIDX = 128                        # gates per dma_gather (1 slot)
NGATHER = GPC // GIDX             # gathers per operand per core = 8
F32 = mybir.dt.float32
BF16 = mybir.dt.bfloat16
I16 = mybir.dt.int16
NP_BF16 = mybir.dt.np(BF16)

# coefficient matrix: op i -> (c0, c1, c2, c3) with value c0 + c1*a + c2*b
# + c3*a*b; rows follow the reference's 16-op ordering.
_OP2AFF = np.array([
    [0, 0, 0, 0],     # false
    [0, 0, 0, 1],     # a and b
    [0, 1, 0, -1],    # a and not b
    [0, 1, 0, 0],     # a
    [0, 0, 1, -1],    # not a and b
    [0, 0, 1, 0],     # b
    [0, 1, 1, -2],    # xor
    [0, 1, 1, -1],    # or
    [1, -1, -1, 1],   # nor
    [1, -1, -1, 2],   # xnor
    [1, 0, -1, 0],    # not b
    [1, 0, -1, 1],    # a or not b
    [1, -1, 0, 0],    # not a
    [1, -1, 0, 1],    # not a or b
    [1, 0, 0, -1],    # nand
    [1, 0, 0, 0],     # true
], dtype=np.float32)

_compiled = {}


def _build_nc(reps=1):
    """Build the per-core program. `reps` unrolls the whole kernel body
    that many times (all reps recompute the identical full output) —
    used by the timing harness to amortize per-dispatch overhead; the
    functional kernel() path uses reps=1."""
    nc = bacc.Bacc("TRN2", target_bir_lowering=False, debug=False,
                   num_devices=N_CORES, num_swdge_queues=2)
    xT = nc.dram_tensor("xT", [IN_DIM, BATCH], BF16, kind="ExternalInput")
    iw = GIDX // 16                   # idx free-cols per gather
    ia_d = nc.dram_tensor("ia", [128, NGATHER * iw], I16, kind="ExternalInput")
    ib_d = nc.dram_tensor("ib", [128, NGATHER * iw], I16, kind="ExternalInput")
    cf_d = nc.dram_tensor("cf", [4, 128, SLOTS], F32, kind="ExternalInput")
    outT = nc.dram_tensor("outT", [GPC, BATCH], BF16, kind="ExternalOutput")

    with tile.TileContext(nc) as tc, ExitStack() as ctx:
        const = ctx.enter_context(tc.tile_pool(name="const", bufs=1))
        # one slot per gather, deep buffering so gathers run well ahead of
        # compute without slot-reuse stalls at rep boundaries
        pa = ctx.enter_context(tc.tile_pool(name="a", bufs=8))
        pb = ctx.enter_context(tc.tile_pool(name="b", bufs=8))
        pu = ctx.enter_context(tc.tile_pool(name="u", bufs=2))
        pw = ctx.enter_context(tc.tile_pool(name="w", bufs=2))
        po = ctx.enter_context(tc.tile_pool(name="o", bufs=3))

        # index tiles first: the gathers depend on them, the coefficient
        # tiles are only needed once compute starts.
        ia = const.tile([128, NGATHER * iw], I16, tag="ia")
        ib = const.tile([128, NGATHER * iw], I16, tag="ib")
        nc.sync.dma_start(ia[:], ia_d.ap()[:])
        nc.sync.dma_start(ib[:], ib_d.ap()[:])
        cs = []
        for k in range(4):
            ck = const.tile([128, SLOTS], F32, tag=f"c{k}")
            nc.sync.dma_start(ck[:], cf_d.ap()[k])
            cs.append(ck)
        C0, C1, C2, C3 = cs

        for rep in range(reps):
            for s in range(NGATHER):
                A = pa.tile([128, 1, BATCH], BF16, tag="A")
                nc.gpsimd.dma_gather(A[:], xT.ap()[:],
                                     ia[:, s * iw:(s + 1) * iw],
                                     GIDX, GIDX, BATCH)
                B = pb.tile([128, 1, BATCH], BF16, tag="B")
                nc.gpsimd.dma_gather(B[:], xT.ap()[:],
                                     ib[:, s * iw:(s + 1) * iw],
                                     GIDX, GIDX, BATCH, queue_num=1)
                a2, b2 = A[:, 0, :], B[:, 0, :]
                u = pu.tile([128, BATCH], BF16, tag="u")
                nc.vector.tensor_scalar(u[:], a2, C3[:, s:s + 1],
                                        C2[:, s:s + 1],
                                        mybir.AluOpType.mult,
                                        mybir.AluOpType.add)
                w = pw.tile([128, BATCH], BF16, tag="w")
                nc.scalar.activation(w[:], a2,
                                     mybir.ActivationFunctionType.Identity,
                                     bias=C0[:, s:s + 1],
                                     scale=C1[:, s:s + 1])
                nc.vector.tensor_mul(u[:], u[:], b2)
                o = po.tile([128, BATCH], BF16, tag="o")
                nc.vector.tensor_add(o[:], u[:], w[:])
                nc.sync.dma_start(outT.ap()[s * 128:(s + 1) * 128, :], o[:])

    nc.compile()
    return nc


def _wrap_idx(conn_shard: np.ndarray) -> np.ndarray:
    """SWDGE index wrapping: per gather of GIDX gates, list position i sits
    at partition i%16, free slot i//16, replicated across the 8 Q7 core
    partition blocks of 16."""
    iw = GIDX // 16
    w = np.empty((128, NGATHER * iw), np.int16)
    for gc in range(NGATHER):
        blk = conn_shard[gc * GIDX:(gc + 1) * GIDX].reshape(iw, 16).T
        w[:, gc * iw:(gc + 1) * iw] = np.tile(blk, (8, 1))
    return w


def make_in_maps(x, weights, conn_a, conn_b):
    x = np.asarray(x, dtype=np.float32)
    weights = np.asarray(weights, dtype=np.float32)
    ca = np.asarray(conn_a).astype(np.int16)
    cb = np.asarray(conn_b).astype(np.int16)
    # softmax(weights) @ affine-coefficient matrix -> [OUT_DIM, 4] f32
    e = np.exp(weights - weights.max(axis=1, keepdims=True))
    sm = e / e.sum(axis=1, keepdims=True)
    cofs = sm @ _OP2AFF                                  # [OUT_DIM, 4]
    xT = np.ascontiguousarray(x.T.astype(NP_BF16))       # [4096, 4096] bf16
    in_maps = []
    perms = []
    for c in range(N_CORES):
        g0, g1 = c * GPC, (c + 1) * GPC
        # sort this core's gates by conn_a: ascending gather addresses are
        # HBM-friendlier; pure host-side permutation, undone in assemble_out
        perm = np.argsort(ca[g0:g1], kind="stable")
        perms.append(perm)
        # position g0 + 128*s + p holds gate perm[128*s + p] -> cf[k, p, s]
        cf = np.ascontiguousarray(
            cofs[g0:g1][perm].reshape(SLOTS, 128, 4).transpose(2, 1, 0))
        in_maps.append({
            "xT": xT,
            "ia": _wrap_idx(ca[g0:g1][perm]),
            "ib": _wrap_idx(cb[g0:g1][perm]),
            "cf": cf.astype(np.float32),
        })
    _compiled["perms"] = perms
    return in_maps


def get_nc(reps=1):
    key = ("nc", reps)
    if key not in _compiled:
        _compiled[key] = _build_nc(reps)
    return _compiled[key]


def assemble_out(results) -> np.ndarray:
    perms = _compiled["perms"]
    out = np.empty((BATCH, OUT_DIM), np.float32)
    for c in range(N_CORES):
        arr = np.asarray(results[c]["outT"])             # [1024, 4096] bf16
        out[:, c * GPC + perms[c]] = arr.T.astype(np.float32)
    return out


def kernel(x, weights, conn_a, conn_b) -> np.ndarray:
    nc = get_nc()
    in_maps = make_in_maps(x, weights, conn_a, conn_b)
    res = run_bass_kernel_spmd(nc, in_maps, core_ids=list(range(N_CORES)))
    return assemble_out(res.results)
